# revision 6
# baseline (speedup 1.0000x reference)
"""AvgPoolingSelfAttention Trainium2 kernel, 8-core head-parallel.

Sharding: B*H = 32 attention instances; each of the 8 cores owns 2 heads
(contiguous 128-column slice of the QKV projections) for both batch items.
Inputs are replicated (hidden states, mask) or column-sharded (weights) on
the host; each core computes its output slice [B, T, 128] independently —
no collectives. Host-side prep is layout-only (transpose/reshape/slice).

On-device per core:
  phase 1: stream hsT [d, t] tiles; Q projection (fp32r matmuls, d-chunk
           accumulated in PSUM); avg-pool (sum of 4 strided slices; the /4
           is folded into Wk/Wv on host).
  phase 2: K/V projections from pooledT; transpose V per head into
           [tk, 64+1] tiles (ones column -> softmax denominator for free).
  phase 3: scores^T [tk, tq] = k.q (K=64 matmuls); exp on ScalarE with
           scale=1/8 and per-partition mask bias fused in; context matmul
           accumulating over tk with the ones column producing sum(exp);
           transpose to natural layout, multiply by reciprocal, DMA out.
"""

import numpy as np

B, T, D = 2, 4096, 1024
H, DH, KP = 16, 64, 4
TK = T // KP            # 1024 pooled keys per batch
NCORES = 8
HPC = H // NCORES       # heads per core
OC = HPC * DH           # 128 projection columns per core
P = 128
NDCH = D // P           # 8 contraction chunks

_CACHE = {}


def _build_nc():
    from contextlib import ExitStack

    import concourse.bacc as bacc
    import concourse.mybir as mybir
    import concourse.tile as tile

    F32 = mybir.dt.float32
    F32R = mybir.dt.float32r
    AF = mybir.ActivationFunctionType

    nc = bacc.Bacc()
    hsT = nc.declare_dram_parameter("hsT", [B, D, T], F32R, isOutput=False)
    wqt = nc.declare_dram_parameter("wqt", [NDCH, P, OC], F32R, isOutput=False)
    wkt = nc.declare_dram_parameter("wkt", [NDCH, P, OC], F32R, isOutput=False)
    wvt = nc.declare_dram_parameter("wvt", [NDCH, P, OC], F32R, isOutput=False)
    bq_d = nc.declare_dram_parameter("bq", [OC, 1], F32, isOutput=False)
    bk_d = nc.declare_dram_parameter("bk", [OC, 1], F32, isOutput=False)
    bv_d = nc.declare_dram_parameter("bv", [OC, 1], F32, isOutput=False)
    mask_d = nc.declare_dram_parameter("maskr", [B, P, 32], F32, isOutput=False)
    id_d = nc.declare_dram_parameter("ident", [P, P], F32, isOutput=False)
    out_d = nc.declare_dram_parameter("out", [B, T, OC], F32, isOutput=True)

    with tile.TileContext(nc) as tc, ExitStack() as ctx:
        wp = ctx.enter_context(tc.tile_pool(name="weights", bufs=1))
        sp = ctx.enter_context(tc.tile_pool(name="small", bufs=2))
        hp = ctx.enter_context(tc.tile_pool(name="hstream", bufs=2))
        bigp = ctx.enter_context(tc.tile_pool(name="big", bufs=1))
        pp = ctx.enter_context(tc.tile_pool(name="ptmp", bufs=2))
        ep = ctx.enter_context(tc.tile_pool(name="exp", bufs=3))
        otp = ctx.enter_context(tc.tile_pool(name="otile", bufs=2))
        psA = ctx.enter_context(tc.tile_pool(name="psA", bufs=2, space="PSUM"))
        psB = ctx.enter_context(tc.tile_pool(name="psB", bufs=2, space="PSUM"))

        ws = {}
        for name, dram in (("wq", wqt), ("wk", wkt), ("wv", wvt)):
            for c in range(NDCH):
                t = wp.tile([P, OC], F32R, tag=f"{name}{c}", name=f"{name}{c}")
                nc.sync.dma_start(t[:], dram[c])
                ws[name, c] = t
        bias_s = {}
        for name, dram in (("bq", bq_d), ("bk", bk_d), ("bv", bv_d)):
            t = wp.tile([OC, 1], F32, tag=name, name=name)
            nc.sync.dma_start(t[:], dram[:])
            bias_s[name] = t
        id_s = wp.tile([P, P], F32, tag="ident")
        nc.sync.dma_start(id_s[:], id_d[:])

        for b in range(B):
            # --- mask -> per-key additive bias, chunk c on column c ---
            mk = sp.tile([P, 32], F32, tag="mk")
            nc.sync.dma_start(mk[:], mask_d[b])
            mk4 = mk[:].rearrange("p (c j) -> p c j", j=KP)
            t1 = sp.tile([P, 8], F32, tag="mt1")
            nc.vector.tensor_add(t1[:], mk4[:, :, 0], mk4[:, :, 1])
            t2 = sp.tile([P, 8], F32, tag="mt2")
            nc.vector.tensor_add(t2[:], mk4[:, :, 2], mk4[:, :, 3])
            mb = sp.tile([P, 8], F32, tag="mb")
            nc.vector.tensor_add(mb[:], t1[:], t2[:])
            nc.vector.tensor_scalar_min(mb[:], mb[:], 1.0)
            nc.vector.tensor_scalar_mul(mb[:], mb[:], -10000.0)

            q2 = bigp.tile([OC, T], F32R, tag="q2")
            pt = [bigp.tile([P, TK], F32R, tag=f"pt{c}", name=f"pt{c}") for c in range(NDCH)]
            k2 = bigp.tile([OC, TK], F32R, tag="k2")
            v2 = bigp.tile([OC, TK], F32, tag="v2")

            # --- phase 1: Q projection + pooling ---
            for ti in range(T // 512):
                hts = []
                for c in range(NDCH):
                    ht = hp.tile([P, 512], F32R, tag=f"hs{c}")
                    nc.sync.dma_start(
                        ht[:], hsT[b, c * P:(c + 1) * P, ti * 512:(ti + 1) * 512]
                    )
                    hts.append(ht)
                qp = psA.tile([OC, 512], F32, tag="psA")
                for c in range(NDCH):
                    nc.tensor.matmul(
                        qp[:],
                        ws["wq", c][:],
                        hts[c][:],
                        start=(c == 0),
                        stop=(c == NDCH - 1),
                    )
                nc.vector.tensor_scalar_add(
                    q2[:, ti * 512:(ti + 1) * 512], qp[:], bias_s["bq"][:]
                )
                for c in range(NDCH):
                    h4 = hts[c][:].rearrange("p (t j) -> p t j", j=KP)
                    pa = pp.tile([P, P], F32, tag="pa")
                    nc.gpsimd.tensor_add(pa[:], h4[:, :, 0], h4[:, :, 1])
                    pb = pp.tile([P, P], F32, tag="pb")
                    nc.vector.tensor_add(pb[:], h4[:, :, 2], h4[:, :, 3])
                    nc.vector.tensor_add(
                        pt[c][:, ti * P:(ti + 1) * P], pa[:], pb[:]
                    )

            # --- phase 2: K/V projections + V transpose ---
            for ki in range(TK // 512):
                kp_ = psA.tile([OC, 512], F32, tag="psA")
                for c in range(NDCH):
                    nc.tensor.matmul(
                        kp_[:],
                        ws["wk", c][:],
                        pt[c][:, ki * 512:(ki + 1) * 512],
                        start=(c == 0),
                        stop=(c == NDCH - 1),
                    )
                nc.vector.tensor_scalar_add(
                    k2[:, ki * 512:(ki + 1) * 512], kp_[:], bias_s["bk"][:]
                )
                vp_ = psA.tile([OC, 512], F32, tag="psA")
                for c in range(NDCH):
                    nc.tensor.matmul(
                        vp_[:],
                        ws["wv", c][:],
                        pt[c][:, ki * 512:(ki + 1) * 512],
                        start=(c == 0),
                        stop=(c == NDCH - 1),
                    )
                nc.vector.tensor_scalar_add(
                    v2[:, ki * 512:(ki + 1) * 512], vp_[:], bias_s["bv"][:]
                )
            vhat = [[None] * (TK // P) for _ in range(HPC)]
            for h in range(HPC):
                for j in range(TK // P):
                    vt = psB.tile([P, DH], F32, tag="nat", bufs=2)
                    nc.tensor.transpose(
                        vt[:],
                        v2[h * DH:(h + 1) * DH, j * P:(j + 1) * P],
                        id_s[h * DH:(h + 1) * DH, h * DH:(h + 1) * DH],
                    )
                    vh = sp.tile([P, DH + 1], F32R, tag=f"vh{h}_{j}")
                    nc.vector.tensor_copy(vh[:, 0:DH], vt[:])
                    nc.vector.tensor_scalar(
                        vh[:, DH:DH + 1], vt[:, 0:1], 0.0, 1.0,
                        mybir.AluOpType.mult, mybir.AluOpType.add,
                    )
                    vhat[h][j] = vh

            # --- phase 3: attention ---
            for si in range(T // 1024):
                q0 = si * 1024
                ot = [otp.tile([P, 512], F32, tag=f"ot{half}", name=f"ot{half}") for half in range(2)]
                for h in range(HPC):
                    cx = [psB.tile([DH + 1, 512], F32, tag="cx", name=f"cx{half}", bufs=2)
                          for half in range(2)]
                    for j in range(TK // P):
                        sc = psA.tile([P, 1024], F32, tag="psA")
                        for half in range(2):
                            nc.tensor.matmul(
                                sc[:, half * 512:(half + 1) * 512],
                                k2[h * DH:(h + 1) * DH, j * P:(j + 1) * P],
                                q2[h * DH:(h + 1) * DH,
                                   q0 + half * 512:q0 + (half + 1) * 512],
                                start=True,
                                stop=True,
                            )
                        ex = ep.tile([P, 1024], F32R, tag="exp")
                        nc.scalar.activation(
                            ex[:], sc[:], AF.Exp,
                            bias=mb[:, j:j + 1], scale=1.0 / 8.0,
                        )
                        for half in range(2):
                            nc.tensor.matmul(
                                cx[half][:],
                                vhat[h][j][:],
                                ex[:, half * 512:(half + 1) * 512],
                                start=(j == 0),
                                stop=(j == TK // P - 1),
                            )
                    for half in range(2):
                        cxs = pp.tile([DH + 1, 512], F32, tag="cxs")
                        nc.vector.tensor_copy(cxs[:], cx[half][:])
                        for q4 in range(4):
                            nat = psB.tile([P, DH + 1], F32, tag="nat", bufs=2)
                            nc.tensor.transpose(
                                nat[:],
                                cxs[:, q4 * P:(q4 + 1) * P],
                                id_s[0:DH + 1, 0:DH + 1],
                            )
                            r = pp.tile([P, 1], F32, tag="r")
                            nc.vector.reciprocal(r[:], nat[:, DH:DH + 1])
                            nc.vector.tensor_scalar_mul(
                                ot[half][:, q4 * P + h * DH:q4 * P + h * DH + DH],
                                nat[:, 0:DH],
                                r[:],
                            )
                for half in range(2):
                    for q4 in range(4):
                        r0 = q0 + half * 512 + q4 * P
                        nc.sync.dma_start(
                            out_d[b, r0:r0 + P, :],
                            ot[half][:, q4 * P:(q4 + 1) * P],
                        )

    nc.finalize()
    return nc


def _prep_in_maps(inputs):
    hs = np.ascontiguousarray(np.asarray(inputs["hidden_states"], dtype=np.float32))
    am = np.asarray(inputs["attention_mask"]).reshape(B, T).astype(np.float32)
    Wq = np.asarray(inputs["Wq"], dtype=np.float32)
    Wk = np.asarray(inputs["Wk"], dtype=np.float32)
    Wv = np.asarray(inputs["Wv"], dtype=np.float32)
    bq = np.asarray(inputs["bq"], dtype=np.float32)
    bk = np.asarray(inputs["bk"], dtype=np.float32)
    bv = np.asarray(inputs["bv"], dtype=np.float32)

    hsT = np.ascontiguousarray(hs.transpose(0, 2, 1))          # [B, D, T]
    # mask, bucket-major then chunk-transposed: [b][p][c*4+j], tk = c*128+p
    mr = np.ascontiguousarray(
        am.reshape(B, TK // P, P, KP).transpose(0, 2, 1, 3).reshape(B, P, 32)
    )
    ident = np.eye(P, dtype=np.float32)

    in_maps = []
    for m in range(NCORES):
        sl = slice(OC * m, OC * (m + 1))
        in_maps.append({
            "hsT": hsT,
            "wqt": np.ascontiguousarray(Wq[sl, :].T).reshape(NDCH, P, OC),
            "wkt": np.ascontiguousarray(Wk[sl, :].T * (1.0 / KP)).reshape(NDCH, P, OC),
            "wvt": np.ascontiguousarray(Wv[sl, :].T * (1.0 / KP)).reshape(NDCH, P, OC),
            "bq": bq[sl].reshape(OC, 1).copy(),
            "bk": bk[sl].reshape(OC, 1).copy(),
            "bv": bv[sl].reshape(OC, 1).copy(),
            "maskr": mr,
            "ident": ident,
        })
    return in_maps


def run(inputs, trace=False):
    """Returns (full_output [B, T, D] fp32, exec_time_ns or None)."""
    from concourse.bass_utils import run_bass_kernel_spmd

    if "nc" not in _CACHE:
        _CACHE["nc"] = _build_nc()
    nc = _CACHE["nc"]
    in_maps = _prep_in_maps(inputs)
    res = run_bass_kernel_spmd(nc, in_maps, list(range(NCORES)), trace=trace)
    full = np.empty((B, T, D), dtype=np.float32)
    for m in range(NCORES):
        full[:, :, OC * m:OC * (m + 1)] = res.results[m]["out"]
    return full, res.exec_time_ns


def kernel(**inputs):
    out, _ = run(inputs, trace=False)
    return out


# revision 7
# speedup vs baseline: 1.0011x; 1.0011x over previous
"""AvgPoolingSelfAttention Trainium2 kernel, 8-core head-parallel.

Sharding: B*H = 32 attention instances; each of the 8 cores owns 2 heads
(contiguous 128-column slice of the QKV projections) for both batch items.
Inputs are replicated (hidden states, mask) or column-sharded (weights) on
the host; each core computes its output slice [B, T, 128] independently —
no collectives. Host-side prep is layout-only (transpose/reshape/slice).

On-device per core:
  phase 1: stream hsT [d, t] tiles; Q projection (fp32r matmuls, d-chunk
           accumulated in PSUM); avg-pool (sum of 4 strided slices; the /4
           is folded into Wk/Wv on host).
  phase 2: K/V projections from pooledT; transpose V per head into
           [tk, 64+1] tiles (ones column -> softmax denominator for free).
  phase 3: scores^T [tk, tq] = k.q (K=64 matmuls); exp on ScalarE with
           scale=1/8 and per-partition mask bias fused in; context matmul
           accumulating over tk with the ones column producing sum(exp);
           transpose to natural layout, multiply by reciprocal, DMA out.
"""

import numpy as np

B, T, D = 2, 4096, 1024
H, DH, KP = 16, 64, 4
TK = T // KP            # 1024 pooled keys per batch
NCORES = 8
HPC = H // NCORES       # heads per core
OC = HPC * DH           # 128 projection columns per core
P = 128
NDCH = D // P           # 8 contraction chunks

_CACHE = {}


def _build_nc():
    from contextlib import ExitStack

    import concourse.bacc as bacc
    import concourse.mybir as mybir
    import concourse.tile as tile

    F32 = mybir.dt.float32
    F32R = mybir.dt.float32r
    AF = mybir.ActivationFunctionType

    nc = bacc.Bacc()
    hsT = nc.declare_dram_parameter("hsT", [B, D, T], F32R, isOutput=False)
    wqt = nc.declare_dram_parameter("wqt", [NDCH, P, OC], F32R, isOutput=False)
    wkt = nc.declare_dram_parameter("wkt", [NDCH, P, OC], F32R, isOutput=False)
    wvt = nc.declare_dram_parameter("wvt", [NDCH, P, OC], F32R, isOutput=False)
    bq_d = nc.declare_dram_parameter("bq", [OC, 1], F32, isOutput=False)
    bk_d = nc.declare_dram_parameter("bk", [OC, 1], F32, isOutput=False)
    bv_d = nc.declare_dram_parameter("bv", [OC, 1], F32, isOutput=False)
    mask_d = nc.declare_dram_parameter("maskr", [B, P, 32], F32, isOutput=False)
    id_d = nc.declare_dram_parameter("ident", [P, P], F32, isOutput=False)
    out_d = nc.declare_dram_parameter("out", [B, T, OC], F32, isOutput=True)

    with tile.TileContext(nc) as tc, ExitStack() as ctx:
        wp = ctx.enter_context(tc.tile_pool(name="weights", bufs=1))
        sp = ctx.enter_context(tc.tile_pool(name="small", bufs=2))
        hp = ctx.enter_context(tc.tile_pool(name="hstream", bufs=2))
        bigp = ctx.enter_context(tc.tile_pool(name="big", bufs=1))
        pp = ctx.enter_context(tc.tile_pool(name="ptmp", bufs=2))
        ep = ctx.enter_context(tc.tile_pool(name="exp", bufs=3))
        otp = ctx.enter_context(tc.tile_pool(name="otile", bufs=2))
        psA = ctx.enter_context(tc.tile_pool(name="psA", bufs=2, space="PSUM"))
        psB = ctx.enter_context(tc.tile_pool(name="psB", bufs=2, space="PSUM"))

        ws = {}
        for name, dram in (("wq", wqt), ("wk", wkt), ("wv", wvt)):
            for c in range(NDCH):
                t = wp.tile([P, OC], F32R, tag=f"{name}{c}", name=f"{name}{c}")
                nc.sync.dma_start(t[:], dram[c])
                ws[name, c] = t
        bias_s = {}
        for name, dram in (("bq", bq_d), ("bk", bk_d), ("bv", bv_d)):
            t = wp.tile([OC, 1], F32, tag=name, name=name)
            nc.sync.dma_start(t[:], dram[:])
            bias_s[name] = t
        id_s = wp.tile([P, P], F32, tag="ident")
        nc.sync.dma_start(id_s[:], id_d[:])

        for b in range(B):
            # --- mask -> per-key additive bias, chunk c on column c ---
            mk = sp.tile([P, 32], F32, tag="mk")
            nc.sync.dma_start(mk[:], mask_d[b])
            mk4 = mk[:].rearrange("p (c j) -> p c j", j=KP)
            t1 = sp.tile([P, 8], F32, tag="mt1")
            nc.vector.tensor_add(t1[:], mk4[:, :, 0], mk4[:, :, 1])
            t2 = sp.tile([P, 8], F32, tag="mt2")
            nc.vector.tensor_add(t2[:], mk4[:, :, 2], mk4[:, :, 3])
            mb = sp.tile([P, 8], F32, tag="mb")
            nc.vector.tensor_add(mb[:], t1[:], t2[:])
            nc.vector.tensor_scalar_min(mb[:], mb[:], 1.0)
            nc.vector.tensor_scalar_mul(mb[:], mb[:], -10000.0)

            q2 = bigp.tile([OC, T], F32R, tag="q2", bufs=2)
            pt = [bigp.tile([P, TK], F32R, tag=f"pt{c}", name=f"pt{c}") for c in range(NDCH)]
            k2 = bigp.tile([OC, TK], F32R, tag="k2", bufs=2)
            v2 = bigp.tile([OC, TK], F32, tag="v2")

            # --- phase 1: Q projection + pooling ---
            for ti in range(T // 512):
                hts = []
                for c in range(NDCH):
                    ht = hp.tile([P, 512], F32R, tag=f"hs{c}")
                    nc.sync.dma_start(
                        ht[:], hsT[b, c * P:(c + 1) * P, ti * 512:(ti + 1) * 512]
                    )
                    hts.append(ht)
                qp = psA.tile([OC, 512], F32, tag="psA")
                for c in range(NDCH):
                    nc.tensor.matmul(
                        qp[:],
                        ws["wq", c][:],
                        hts[c][:],
                        start=(c == 0),
                        stop=(c == NDCH - 1),
                    )
                nc.vector.tensor_scalar_add(
                    q2[:, ti * 512:(ti + 1) * 512], qp[:], bias_s["bq"][:]
                )
                for c in range(NDCH):
                    h4 = hts[c][:].rearrange("p (t j) -> p t j", j=KP)
                    pa = pp.tile([P, P], F32, tag="pa")
                    nc.gpsimd.tensor_add(pa[:], h4[:, :, 0], h4[:, :, 1])
                    pb = pp.tile([P, P], F32, tag="pb")
                    nc.vector.tensor_add(pb[:], h4[:, :, 2], h4[:, :, 3])
                    nc.vector.tensor_add(
                        pt[c][:, ti * P:(ti + 1) * P], pa[:], pb[:]
                    )

            # --- phase 2: K/V projections + V transpose ---
            for ki in range(TK // 512):
                kp_ = psA.tile([OC, 512], F32, tag="psA")
                for c in range(NDCH):
                    nc.tensor.matmul(
                        kp_[:],
                        ws["wk", c][:],
                        pt[c][:, ki * 512:(ki + 1) * 512],
                        start=(c == 0),
                        stop=(c == NDCH - 1),
                    )
                nc.vector.tensor_scalar_add(
                    k2[:, ki * 512:(ki + 1) * 512], kp_[:], bias_s["bk"][:]
                )
                vp_ = psA.tile([OC, 512], F32, tag="psA")
                for c in range(NDCH):
                    nc.tensor.matmul(
                        vp_[:],
                        ws["wv", c][:],
                        pt[c][:, ki * 512:(ki + 1) * 512],
                        start=(c == 0),
                        stop=(c == NDCH - 1),
                    )
                nc.vector.tensor_scalar_add(
                    v2[:, ki * 512:(ki + 1) * 512], vp_[:], bias_s["bv"][:]
                )
            vhat = [[None] * (TK // P) for _ in range(HPC)]
            for h in range(HPC):
                for j in range(TK // P):
                    vt = psB.tile([P, DH], F32, tag="nat", bufs=2)
                    nc.tensor.transpose(
                        vt[:],
                        v2[h * DH:(h + 1) * DH, j * P:(j + 1) * P],
                        id_s[h * DH:(h + 1) * DH, h * DH:(h + 1) * DH],
                    )
                    vh = sp.tile([P, DH + 1], F32R, tag=f"vh{h}_{j}")
                    nc.vector.tensor_copy(vh[:, 0:DH], vt[:])
                    nc.vector.tensor_scalar(
                        vh[:, DH:DH + 1], vt[:, 0:1], 0.0, 1.0,
                        mybir.AluOpType.mult, mybir.AluOpType.add,
                    )
                    vhat[h][j] = vh

            # --- phase 3: attention ---
            for si in range(T // 1024):
                q0 = si * 1024
                ot = [otp.tile([P, 512], F32, tag=f"ot{half}", name=f"ot{half}") for half in range(2)]
                for h in range(HPC):
                    cx = [psB.tile([DH + 1, 512], F32, tag="cx", name=f"cx{half}", bufs=2)
                          for half in range(2)]
                    for j in range(TK // P):
                        sc = psA.tile([P, 1024], F32, tag="psA")
                        for half in range(2):
                            nc.tensor.matmul(
                                sc[:, half * 512:(half + 1) * 512],
                                k2[h * DH:(h + 1) * DH, j * P:(j + 1) * P],
                                q2[h * DH:(h + 1) * DH,
                                   q0 + half * 512:q0 + (half + 1) * 512],
                                start=True,
                                stop=True,
                            )
                        ex = ep.tile([P, 1024], F32R, tag="exp")
                        nc.scalar.activation(
                            ex[:], sc[:], AF.Exp,
                            bias=mb[:, j:j + 1], scale=1.0 / 8.0,
                        )
                        for half in range(2):
                            nc.tensor.matmul(
                                cx[half][:],
                                vhat[h][j][:],
                                ex[:, half * 512:(half + 1) * 512],
                                start=(j == 0),
                                stop=(j == TK // P - 1),
                            )
                    for half in range(2):
                        cxs = pp.tile([DH + 1, 512], F32, tag="cxs")
                        nc.vector.tensor_copy(cxs[:], cx[half][:])
                        for q4 in range(4):
                            nat = psB.tile([P, DH + 1], F32, tag="nat", bufs=2)
                            nc.tensor.transpose(
                                nat[:],
                                cxs[:, q4 * P:(q4 + 1) * P],
                                id_s[0:DH + 1, 0:DH + 1],
                            )
                            r = pp.tile([P, 1], F32, tag="r")
                            nc.vector.reciprocal(r[:], nat[:, DH:DH + 1])
                            nc.vector.tensor_scalar_mul(
                                ot[half][:, q4 * P + h * DH:q4 * P + h * DH + DH],
                                nat[:, 0:DH],
                                r[:],
                            )
                for half in range(2):
                    for q4 in range(4):
                        r0 = q0 + half * 512 + q4 * P
                        nc.sync.dma_start(
                            out_d[b, r0:r0 + P, :],
                            ot[half][:, q4 * P:(q4 + 1) * P],
                        )

    nc.finalize()
    return nc


def _prep_in_maps(inputs):
    hs = np.ascontiguousarray(np.asarray(inputs["hidden_states"], dtype=np.float32))
    am = np.asarray(inputs["attention_mask"]).reshape(B, T).astype(np.float32)
    Wq = np.asarray(inputs["Wq"], dtype=np.float32)
    Wk = np.asarray(inputs["Wk"], dtype=np.float32)
    Wv = np.asarray(inputs["Wv"], dtype=np.float32)
    bq = np.asarray(inputs["bq"], dtype=np.float32)
    bk = np.asarray(inputs["bk"], dtype=np.float32)
    bv = np.asarray(inputs["bv"], dtype=np.float32)

    hsT = np.ascontiguousarray(hs.transpose(0, 2, 1))          # [B, D, T]
    # mask, bucket-major then chunk-transposed: [b][p][c*4+j], tk = c*128+p
    mr = np.ascontiguousarray(
        am.reshape(B, TK // P, P, KP).transpose(0, 2, 1, 3).reshape(B, P, 32)
    )
    ident = np.eye(P, dtype=np.float32)

    in_maps = []
    for m in range(NCORES):
        sl = slice(OC * m, OC * (m + 1))
        in_maps.append({
            "hsT": hsT,
            "wqt": np.ascontiguousarray(Wq[sl, :].T).reshape(NDCH, P, OC),
            "wkt": np.ascontiguousarray(Wk[sl, :].T * (1.0 / KP)).reshape(NDCH, P, OC),
            "wvt": np.ascontiguousarray(Wv[sl, :].T * (1.0 / KP)).reshape(NDCH, P, OC),
            "bq": bq[sl].reshape(OC, 1).copy(),
            "bk": bk[sl].reshape(OC, 1).copy(),
            "bv": bv[sl].reshape(OC, 1).copy(),
            "maskr": mr,
            "ident": ident,
        })
    return in_maps


def run(inputs, trace=False):
    """Returns (full_output [B, T, D] fp32, exec_time_ns or None)."""
    from concourse.bass_utils import run_bass_kernel_spmd

    if "nc" not in _CACHE:
        _CACHE["nc"] = _build_nc()
    nc = _CACHE["nc"]
    in_maps = _prep_in_maps(inputs)
    res = run_bass_kernel_spmd(nc, in_maps, list(range(NCORES)), trace=trace)
    full = np.empty((B, T, D), dtype=np.float32)
    for m in range(NCORES):
        full[:, :, OC * m:OC * (m + 1)] = res.results[m]["out"]
    return full, res.exec_time_ns


def kernel(**inputs):
    out, _ = run(inputs, trace=False)
    return out


# revision 8
# speedup vs baseline: 1.0826x; 1.0814x over previous
"""AvgPoolingSelfAttention Trainium2 kernel, 8-core head-parallel.

Sharding: B*H = 32 attention instances; each of the 8 cores owns 2 heads
(contiguous 128-column slice of the QKV projections) for both batch items.
Inputs are replicated (hidden states, mask) or column-sharded (weights) on
the host; each core computes its output slice [B, T, 128] independently —
no collectives. Host-side prep is layout-only (transpose/reshape/slice).

On-device per core:
  phase 1: stream hsT [d, t] tiles; Q projection (fp32r matmuls, d-chunk
           accumulated in PSUM); avg-pool (sum of 4 strided slices; the /4
           is folded into Wk/Wv on host).
  phase 2: K/V projections from pooledT; transpose V per head into
           [tk, 64+1] tiles (ones column -> softmax denominator for free).
  phase 3: scores^T [tk, tq] = k.q (K=64 matmuls); exp on ScalarE with
           scale=1/8 and per-partition mask bias fused in; context matmul
           accumulating over tk with the ones column producing sum(exp);
           transpose to natural layout, multiply by reciprocal, DMA out.
"""

import numpy as np

B, T, D = 2, 4096, 1024
H, DH, KP = 16, 64, 4
TK = T // KP            # 1024 pooled keys per batch
NCORES = 8
HPC = H // NCORES       # heads per core
OC = HPC * DH           # 128 projection columns per core
P = 128
NDCH = D // P           # 8 contraction chunks

_CACHE = {}


def _build_nc():
    from contextlib import ExitStack

    import concourse.bacc as bacc
    import concourse.mybir as mybir
    import concourse.tile as tile

    F32 = mybir.dt.float32
    F32R = mybir.dt.float32r
    AF = mybir.ActivationFunctionType

    nc = bacc.Bacc()
    hsT = nc.declare_dram_parameter("hsT", [B, NDCH, T // 512, P, 512], F32R, isOutput=False)
    wqt = nc.declare_dram_parameter("wqt", [NDCH, P, OC], F32R, isOutput=False)
    wkt = nc.declare_dram_parameter("wkt", [NDCH, P, OC], F32R, isOutput=False)
    wvt = nc.declare_dram_parameter("wvt", [NDCH, P, OC], F32R, isOutput=False)
    bq_d = nc.declare_dram_parameter("bq", [OC, 1], F32, isOutput=False)
    bk_d = nc.declare_dram_parameter("bk", [OC, 1], F32, isOutput=False)
    bv_d = nc.declare_dram_parameter("bv", [OC, 1], F32, isOutput=False)
    mask_d = nc.declare_dram_parameter("maskr", [B, P, 32], F32, isOutput=False)
    id_d = nc.declare_dram_parameter("ident", [P, P], F32, isOutput=False)
    out_d = nc.declare_dram_parameter("out", [B, T, OC], F32, isOutput=True)

    with tile.TileContext(nc) as tc, ExitStack() as ctx:
        wp = ctx.enter_context(tc.tile_pool(name="weights", bufs=1))
        sp = ctx.enter_context(tc.tile_pool(name="small", bufs=2))
        hp = ctx.enter_context(tc.tile_pool(name="hstream", bufs=2))
        bigp = ctx.enter_context(tc.tile_pool(name="big", bufs=1))
        pp = ctx.enter_context(tc.tile_pool(name="ptmp", bufs=2))
        ep = ctx.enter_context(tc.tile_pool(name="exp", bufs=3))
        otp = ctx.enter_context(tc.tile_pool(name="otile", bufs=2))
        psA = ctx.enter_context(tc.tile_pool(name="psA", bufs=2, space="PSUM"))
        psB = ctx.enter_context(tc.tile_pool(name="psB", bufs=2, space="PSUM"))

        ws = {}
        for name, dram in (("wq", wqt), ("wk", wkt), ("wv", wvt)):
            for c in range(NDCH):
                t = wp.tile([P, OC], F32R, tag=f"{name}{c}", name=f"{name}{c}")
                nc.sync.dma_start(t[:], dram[c])
                ws[name, c] = t
        bias_s = {}
        for name, dram in (("bq", bq_d), ("bk", bk_d), ("bv", bv_d)):
            t = wp.tile([OC, 1], F32, tag=name, name=name)
            nc.sync.dma_start(t[:], dram[:])
            bias_s[name] = t
        id_s = wp.tile([P, P], F32, tag="ident")
        nc.sync.dma_start(id_s[:], id_d[:])

        for b in range(B):
            # --- mask -> per-key additive bias, chunk c on column c ---
            mk = sp.tile([P, 32], F32, tag="mk")
            nc.sync.dma_start(mk[:], mask_d[b])
            mk4 = mk[:].rearrange("p (c j) -> p c j", j=KP)
            t1 = sp.tile([P, 8], F32, tag="mt1")
            nc.vector.tensor_add(t1[:], mk4[:, :, 0], mk4[:, :, 1])
            t2 = sp.tile([P, 8], F32, tag="mt2")
            nc.vector.tensor_add(t2[:], mk4[:, :, 2], mk4[:, :, 3])
            mb = sp.tile([P, 8], F32, tag="mb")
            nc.vector.tensor_add(mb[:], t1[:], t2[:])
            nc.vector.tensor_scalar_min(mb[:], mb[:], 1.0)
            nc.vector.tensor_scalar_mul(mb[:], mb[:], -10000.0)

            q2 = bigp.tile([OC, T], F32R, tag="q2", bufs=2)
            pt = [bigp.tile([P, TK], F32R, tag=f"pt{c}", name=f"pt{c}") for c in range(NDCH)]
            k2 = bigp.tile([OC, TK], F32R, tag="k2", bufs=2)
            v2 = bigp.tile([OC, TK], F32, tag="v2")

            # --- phase 1: Q projection + pooling ---
            for ti in range(T // 512):
                hts = []
                for c in range(NDCH):
                    ht = hp.tile([P, 512], F32R, tag=f"hs{c}")
                    nc.sync.dma_start(ht[:], hsT[b, c, ti])
                    hts.append(ht)
                qp = psA.tile([OC, 512], F32, tag="ps1")
                for c in range(NDCH):
                    nc.tensor.matmul(
                        qp[:],
                        ws["wq", c][:],
                        hts[c][:],
                        start=(c == 0),
                        stop=(c == NDCH - 1),
                    )
                nc.vector.tensor_scalar_add(
                    q2[:, ti * 512:(ti + 1) * 512], qp[:], bias_s["bq"][:]
                )
                for c in range(NDCH):
                    h4 = hts[c][:].rearrange("p (t j) -> p t j", j=KP)
                    pa = pp.tile([P, P], F32, tag="pa")
                    nc.gpsimd.tensor_add(pa[:], h4[:, :, 0], h4[:, :, 1])
                    pb = pp.tile([P, P], F32, tag="pb")
                    nc.vector.tensor_add(pb[:], h4[:, :, 2], h4[:, :, 3])
                    nc.vector.tensor_add(
                        pt[c][:, ti * P:(ti + 1) * P], pa[:], pb[:]
                    )

            # --- phase 2: K/V projections + V transpose ---
            for ki in range(TK // 512):
                kp_ = psA.tile([OC, 512], F32, tag="ps1")
                for c in range(NDCH):
                    nc.tensor.matmul(
                        kp_[:],
                        ws["wk", c][:],
                        pt[c][:, ki * 512:(ki + 1) * 512],
                        start=(c == 0),
                        stop=(c == NDCH - 1),
                    )
                nc.vector.tensor_scalar_add(
                    k2[:, ki * 512:(ki + 1) * 512], kp_[:], bias_s["bk"][:]
                )
                vp_ = psA.tile([OC, 512], F32, tag="ps1")
                for c in range(NDCH):
                    nc.tensor.matmul(
                        vp_[:],
                        ws["wv", c][:],
                        pt[c][:, ki * 512:(ki + 1) * 512],
                        start=(c == 0),
                        stop=(c == NDCH - 1),
                    )
                nc.vector.tensor_scalar_add(
                    v2[:, ki * 512:(ki + 1) * 512], vp_[:], bias_s["bv"][:]
                )
            vhat = [[None] * (TK // P) for _ in range(HPC)]
            for h in range(HPC):
                for j in range(TK // P):
                    vt = psA.tile([P, DH], F32, tag="ps1")
                    nc.tensor.transpose(
                        vt[:],
                        v2[h * DH:(h + 1) * DH, j * P:(j + 1) * P],
                        id_s[h * DH:(h + 1) * DH, h * DH:(h + 1) * DH],
                    )
                    vh = sp.tile([P, DH + 1], F32R, tag=f"vh{h}_{j}")
                    nc.vector.tensor_copy(vh[:, 0:DH], vt[:])
                    nc.vector.tensor_scalar(
                        vh[:, DH:DH + 1], vt[:, 0:1], 0.0, 1.0,
                        mybir.AluOpType.mult, mybir.AluOpType.add,
                    )
                    vhat[h][j] = vh

            # --- phase 3: attention ---
            for si in range(T // 1024):
                q0 = si * 1024
                ot = [otp.tile([P, 512], F32, tag=f"ot{half}", name=f"ot{half}") for half in range(2)]
                for h in range(HPC):
                    cx = [psB.tile([DH + 1, 512], F32, tag="cx", name=f"cx{half}", bufs=2)
                          for half in range(2)]
                    for j in range(TK // P):
                        sc = psA.tile([P, 1024], F32, tag="sc")
                        for half in range(2):
                            nc.tensor.matmul(
                                sc[:, half * 512:(half + 1) * 512],
                                k2[h * DH:(h + 1) * DH, j * P:(j + 1) * P],
                                q2[h * DH:(h + 1) * DH,
                                   q0 + half * 512:q0 + (half + 1) * 512],
                                start=True,
                                stop=True,
                            )
                        ex = ep.tile([P, 1024], F32R, tag="exp")
                        nc.scalar.activation(
                            ex[:], sc[:], AF.Exp,
                            bias=mb[:, j:j + 1], scale=1.0 / 8.0,
                        )
                        for half in range(2):
                            nc.tensor.matmul(
                                cx[half][:],
                                vhat[h][j][:],
                                ex[:, half * 512:(half + 1) * 512],
                                start=(j == 0),
                                stop=(j == TK // P - 1),
                            )
                    for half in range(2):
                        cxs = pp.tile([DH + 1, 512], F32, tag="cxs")
                        nc.vector.tensor_copy(cxs[:], cx[half][:])
                        for q4 in range(4):
                            nat = psB.tile([P, DH + 1], F32, tag="cx", bufs=2)
                            nc.tensor.transpose(
                                nat[:],
                                cxs[:, q4 * P:(q4 + 1) * P],
                                id_s[0:DH + 1, 0:DH + 1],
                            )
                            r = pp.tile([P, 1], F32, tag="r")
                            nc.vector.reciprocal(r[:], nat[:, DH:DH + 1])
                            nc.vector.tensor_scalar_mul(
                                ot[half][:, q4 * P + h * DH:q4 * P + h * DH + DH],
                                nat[:, 0:DH],
                                r[:],
                            )
                for half in range(2):
                    for q4 in range(4):
                        r0 = q0 + half * 512 + q4 * P
                        nc.sync.dma_start(
                            out_d[b, r0:r0 + P, :],
                            ot[half][:, q4 * P:(q4 + 1) * P],
                        )

    nc.finalize()
    return nc


def _prep_in_maps(inputs):
    hs = np.ascontiguousarray(np.asarray(inputs["hidden_states"], dtype=np.float32))
    am = np.asarray(inputs["attention_mask"]).reshape(B, T).astype(np.float32)
    Wq = np.asarray(inputs["Wq"], dtype=np.float32)
    Wk = np.asarray(inputs["Wk"], dtype=np.float32)
    Wv = np.asarray(inputs["Wv"], dtype=np.float32)
    bq = np.asarray(inputs["bq"], dtype=np.float32)
    bk = np.asarray(inputs["bk"], dtype=np.float32)
    bv = np.asarray(inputs["bv"], dtype=np.float32)

    hsT = np.ascontiguousarray(
        hs.transpose(0, 2, 1).reshape(B, NDCH, P, T // 512, 512).transpose(0, 1, 3, 2, 4)
    )  # [B, c, ti, 128, 512] — each [128, 512] tile contiguous
    # mask, bucket-major then chunk-transposed: [b][p][c*4+j], tk = c*128+p
    mr = np.ascontiguousarray(
        am.reshape(B, TK // P, P, KP).transpose(0, 2, 1, 3).reshape(B, P, 32)
    )
    ident = np.eye(P, dtype=np.float32)

    in_maps = []
    for m in range(NCORES):
        sl = slice(OC * m, OC * (m + 1))
        in_maps.append({
            "hsT": hsT,
            "wqt": np.ascontiguousarray(Wq[sl, :].T).reshape(NDCH, P, OC),
            "wkt": np.ascontiguousarray(Wk[sl, :].T * (1.0 / KP)).reshape(NDCH, P, OC),
            "wvt": np.ascontiguousarray(Wv[sl, :].T * (1.0 / KP)).reshape(NDCH, P, OC),
            "bq": bq[sl].reshape(OC, 1).copy(),
            "bk": bk[sl].reshape(OC, 1).copy(),
            "bv": bv[sl].reshape(OC, 1).copy(),
            "maskr": mr,
            "ident": ident,
        })
    return in_maps


def run(inputs, trace=False):
    """Returns (full_output [B, T, D] fp32, exec_time_ns or None)."""
    from concourse.bass_utils import run_bass_kernel_spmd

    if "nc" not in _CACHE:
        _CACHE["nc"] = _build_nc()
    nc = _CACHE["nc"]
    in_maps = _prep_in_maps(inputs)
    res = run_bass_kernel_spmd(nc, in_maps, list(range(NCORES)), trace=trace)
    full = np.empty((B, T, D), dtype=np.float32)
    for m in range(NCORES):
        full[:, :, OC * m:OC * (m + 1)] = res.results[m]["out"]
    return full, res.exec_time_ns


def kernel(**inputs):
    out, _ = run(inputs, trace=False)
    return out
